# revision 7
# baseline (speedup 1.0000x reference)
"""AAUpsample1d Trainium2 kernel.

Reference computation (per batch element):
  y   = conv_transpose1d(x, conv_w, stride=2, k=3) + conv_b        # [512, 8192]
  y   = depthwise_conv1d(y, aa_kernel, k=17, same)                 # anti-alias
  out = proj_w @ y + proj_b                                        # 1x1 projection

Algebraic restructuring used here:
  * The depthwise AA filter applies the SAME 17-tap kernel to every channel, so
    it commutes with the 1x1 channel projection:  proj(AA(y)) == AA(proj(y)).
  * The stride-2 transposed conv splits into two polyphase 1x1 convs.  Folding
    the projection in:  M_k = proj_w @ conv_w[:,:,k]  gives
        z[2u]   = M1 @ x[u]
        z[2u+1] = M0 @ x[u] + M2 @ x[u+1]
        out     = AA(z) + (sum(aa) * proj_w @ conv_b + proj_b)
    which removes the 1x1 projection matmul entirely.
  * AA(z) runs on the TensorEngine as banded-Toeplitz matmuls:  z is produced
    with length-on-partitions (overlapping tiles of 128 u-positions advancing
    by 120, i.e. a +/-4 halo), then  out_tile[c, l] = ze.T @ R_e + zo.T @ R_o
    where R_e/R_o[u, l] place aa taps on diagonals.  One matmul pair per tile
    performs the AA filter, the transpose back to channel-major layout, and
    the even/odd phase interleave simultaneously.

Sharding: pure data-parallel, one batch element per NeuronCore (B=8 = n_cores).
"""

import numpy as np
import ml_dtypes

import concourse.bass as bass
import concourse.mybir as mybir
import concourse.tile as tile
from concourse import bacc
from concourse.bass_utils import run_bass_kernel_spmd

BF16 = ml_dtypes.bfloat16

B, D, L = 8, 512, 4096
LOUT = 2 * L
KSIZE = 17
STRIDE = 120                      # u-positions of fresh output per z tile
HALO = 4                          # +/- halo rows in each z tile
NJ = (L + STRIDE - 1) // STRIDE   # 35 tiles
RCOLS = 2 * STRIDE                # 240 interleaved output columns per tile
XPAD_L = 4                        # left zero pad (covers u < 0 halo)
XCOLS = XPAD_L + L + 132          # 4232: right pad so every slice is in-bounds
NCC = D // 128                    # 4 channel chunks

_CACHE = {}


def _build_bass():
    nc = bacc.Bacc("TRN2", target_bir_lowering=False)
    f32 = mybir.dt.float32
    bf16 = mybir.dt.bfloat16

    x_d = nc.dram_tensor("x", [D, XCOLS], bf16, kind="ExternalInput")
    mt_d = nc.dram_tensor("mt", [D, 3, D], bf16, kind="ExternalInput")
    rt_d = nc.dram_tensor("rt", [128, 2, RCOLS], bf16, kind="ExternalInput")
    bias_d = nc.dram_tensor("bias", [D], f32, kind="ExternalInput")
    out_d = nc.dram_tensor("out", [D, LOUT], f32, kind="ExternalOutput")

    with tile.TileContext(nc) as tc:
        with (
            tc.tile_pool(name="const", bufs=1) as cpool,
            tc.tile_pool(name="zsb", bufs=3) as zpool,
            tc.tile_pool(name="osb", bufs=6) as opool,
            tc.tile_pool(name="zmm", bufs=4, space="PSUM") as zmm,
            tc.tile_pool(name="aa", bufs=4, space="PSUM") as aamm,
        ):
            # ---- constants / inputs ----
            xt = cpool.tile([128, NCC, XCOLS], bf16, name="xt")
            for o in range(NCC):
                nc.sync.dma_start(xt[:, o], x_d[128 * o:128 * (o + 1), :])

            mts = cpool.tile([128, NCC, 3, D], bf16, name="mts")
            for o in range(NCC):
                nc.sync.dma_start(mts[:, o], mt_d[128 * o:128 * (o + 1)])

            rts = cpool.tile([128, 2, RCOLS], bf16, name="rts")
            nc.sync.dma_start(rts[:], rt_d[:])

            biast = cpool.tile([128, NCC], f32, name="biast")
            nc.sync.dma_start(biast[:], bias_d.rearrange("(o p) -> p o", p=128))

            zs = [None] * NJ
            pair_psum = {}

            def emit_main(J):
                # z tile J covers u in [120J-4, 120J+124); SBUF col = u + XPAD_L
                c0 = STRIDE * J  # == 120J - 4 + XPAD_L
                ze_ps = zmm.tile([128, D], f32, tag="zmm", name="ze_ps")
                for o in range(NCC):
                    nc.tensor.matmul(
                        ze_ps, lhsT=xt[:, o, c0:c0 + 128], rhs=mts[:, o, 1],
                        start=(o == 0), stop=(o == NCC - 1),
                    )
                zo_ps = zmm.tile([128, D], f32, tag="zmm", name="zo_ps")
                for o in range(NCC):
                    nc.tensor.matmul(
                        zo_ps, lhsT=xt[:, o, c0:c0 + 128], rhs=mts[:, o, 0],
                        start=(o == 0), stop=False,
                    )
                for o in range(NCC):
                    nc.tensor.matmul(
                        zo_ps, lhsT=xt[:, o, c0 + 1:c0 + 129], rhs=mts[:, o, 2],
                        start=False, stop=(o == NCC - 1),
                    )
                ze_sb = zpool.tile([128, D], bf16, tag="ze", name="ze_sb")
                zo_sb = zpool.tile([128, D], bf16, tag="zo", name="zo_sb")
                nc.vector.tensor_copy(out=ze_sb[:], in_=ze_ps[:])
                nc.vector.tensor_copy(out=zo_sb[:], in_=zo_ps[:])
                zs[J] = (ze_sb, zo_sb)

            def emit_aa(J):
                p, half = divmod(J, 2)
                if half == 0:
                    pair_psum[p] = [
                        aamm.tile([128, 512], f32, tag="aa", name=f"aa_ps{cc}")
                        for cc in range(NCC)
                    ]
                ze_sb, zo_sb = zs[J]
                for cc in range(NCC):
                    dst = pair_psum[p][cc][:, RCOLS * half:RCOLS * half + RCOLS]
                    nc.tensor.matmul(
                        dst, lhsT=ze_sb[:, 128 * cc:128 * (cc + 1)],
                        rhs=rts[:, 0], start=True, stop=False,
                    )
                    nc.tensor.matmul(
                        dst, lhsT=zo_sb[:, 128 * cc:128 * (cc + 1)],
                        rhs=rts[:, 1], start=False, stop=True,
                    )
                zs[J] = None
                if half == 1 or J == NJ - 1:
                    ncols = min(2 * RCOLS, LOUT - 2 * RCOLS * p)
                    for cc in range(NCC):
                        osb = opool.tile([128, 2 * RCOLS], f32, tag="osb", name="osb")
                        nc.scalar.activation(
                            osb[:, :ncols], pair_psum[p][cc][:, :ncols],
                            mybir.ActivationFunctionType.Identity,
                            bias=biast[:, cc:cc + 1], scale=1.0,
                        )
                        nc.sync.dma_start(
                            out_d[128 * cc:128 * (cc + 1),
                                  2 * RCOLS * p:2 * RCOLS * p + ncols],
                            osb[:, :ncols],
                        )
                    del pair_psum[p]

            # software-pipelined emission: AA(J-1) after main(J) so the PE
            # never waits on the DVE z copies
            for J in range(NJ):
                emit_main(J)
                if J >= 1:
                    emit_aa(J - 1)
            emit_aa(NJ - 1)

    nc.compile()
    return nc


def _host_weights(conv_w, conv_b, aa_kernel, proj_w, proj_b):
    aa = np.asarray(aa_kernel, np.float32)
    proj_w = np.asarray(proj_w, np.float32)
    # fold the projection into the three polyphase matrices
    m = [proj_w @ np.asarray(conv_w, np.float32)[:, :, k] for k in range(3)]
    mt_np = np.stack([mk.T for mk in m], axis=1).astype(BF16)  # [ic, 3, oc]

    u = np.arange(128)[:, None]
    l = np.arange(RCOLS)[None, :]
    te = 2 * u - l
    to = 2 * u - l + 1
    r_e = np.where((te >= 0) & (te < KSIZE), aa[np.clip(te, 0, KSIZE - 1)], 0.0)
    r_o = np.where((to >= 0) & (to < KSIZE), aa[np.clip(to, 0, KSIZE - 1)], 0.0)
    rt_np = np.stack([r_e, r_o], axis=1).astype(BF16)  # [128, 2, 240]

    bias_np = (aa.sum() * (proj_w @ np.asarray(conv_b, np.float32))
               + np.asarray(proj_b, np.float32)).astype(np.float32)
    return mt_np, rt_np, bias_np


def kernel(x, conv_w, conv_b, aa_kernel, proj_w, proj_b):
    if "nc" not in _CACHE:
        _CACHE["nc"] = _build_bass()
    nc = _CACHE["nc"]

    mt_np, rt_np, bias_np = _host_weights(conv_w, conv_b, aa_kernel, proj_w, proj_b)
    x = np.asarray(x, np.float32)
    xpad = np.zeros((B, D, XCOLS), BF16)
    xpad[:, :, XPAD_L:XPAD_L + L] = x.astype(BF16)
    in_maps = [
        {"x": xpad[b], "mt": mt_np, "rt": rt_np, "bias": bias_np}
        for b in range(B)
    ]
    res = run_bass_kernel_spmd(nc, in_maps, core_ids=list(range(B)))
    _CACHE["last_results"] = res
    return np.stack([r["out"] for r in res.results], axis=0)


# revision 65
# speedup vs baseline: 1.1350x; 1.1350x over previous
"""AAUpsample1d Trainium2 kernel.

Reference computation (per batch element):
  y   = conv_transpose1d(x, conv_w, stride=2, k=3) + conv_b        # [512, 8192]
  y   = depthwise_conv1d(y, aa_kernel, k=17, same)                 # anti-alias
  out = proj_w @ y + proj_b                                        # 1x1 projection

Algebraic restructuring used here:
  * The depthwise AA filter applies the SAME 17-tap kernel to every channel, so
    it commutes with the 1x1 channel projection:  proj(AA(y)) == AA(proj(y)).
  * The stride-2 transposed conv splits into two polyphase 1x1 convs.  Folding
    the projection in:  M_k = proj_w @ conv_w[:,:,k]  gives
        z[2u]   = M1 @ x[u]
        z[2u+1] = M0 @ x[u] + M2 @ x[u+1]
        out     = AA(z) + (sum(aa) * proj_w @ conv_b + proj_b)
    which removes the 1x1 projection matmul entirely.
  * AA(z) runs on the TensorEngine as banded-Toeplitz matmuls:  z is produced
    with length-on-partitions (overlapping tiles of 128 u-positions advancing
    by 120, i.e. a +/-4 halo), then  out_tile[c, l] = ze.T @ R_e + zo.T @ R_o
    where R_e/R_o[u, l] place aa taps on diagonals.  One matmul pair per tile
    performs the AA filter, the transpose back to channel-major layout, and
    the even/odd phase interleave simultaneously.

Sharding: pure data-parallel, one batch element per NeuronCore (B=8 = n_cores).
"""

import numpy as np
import ml_dtypes

import concourse.bass as bass
import concourse.mybir as mybir
import concourse.tile as tile
from concourse import bacc
from concourse.bass_utils import run_bass_kernel_spmd

BF16 = ml_dtypes.bfloat16

B, D, L = 8, 512, 4096
LOUT = 2 * L
KSIZE = 17
STRIDE = 120                      # u-positions of fresh output per z tile
HALO = 4                          # +/- halo rows in each z tile
NJ = (L + STRIDE - 1) // STRIDE   # 35 tiles
RCOLS = 2 * STRIDE                # 240 interleaved output columns per tile
XPAD_L = 4                        # left zero pad (covers u < 0 halo)
XCOLS = XPAD_L + L + 132          # 4232: right pad so every slice is in-bounds
NCC = D // 128                    # 4 channel chunks
# x segments: smaller first segment shrinks the bytes gating J0's completion
SEGSTARTS = [0, 4, 11, 18, 25, 32]            # first J of each segment
NSEG = len(SEGSTARTS)
_SEGENDS = SEGSTARTS[1:] + [NJ]

_CACHE = {}


def _build_bass():
    nc = bacc.Bacc("TRN2", target_bir_lowering=False)
    f32 = mybir.dt.float32
    bf16 = mybir.dt.bfloat16

    x_d = nc.dram_tensor("x", [D, XCOLS], bf16, kind="ExternalInput")
    mt_d = nc.dram_tensor("mt", [D, 3, D], bf16, kind="ExternalInput")
    rt_d = nc.dram_tensor("rt", [128, 2, RCOLS], bf16, kind="ExternalInput")
    bias_d = nc.dram_tensor("bias", [D], f32, kind="ExternalInput")
    out_d = nc.dram_tensor("out", [D, LOUT], f32, kind="ExternalOutput")

    with tile.TileContext(nc) as tc:
        with (
            tc.tile_pool(name="const", bufs=1) as cpool,
            tc.tile_pool(name="zsb", bufs=2) as zpool,
            tc.tile_pool(name="osb", bufs=3) as opool,
            tc.tile_pool(name="zmm", bufs=4, space="PSUM") as zmm,
            tc.tile_pool(name="aa", bufs=4, space="PSUM") as aamm,
        ):
            # ---- constants / inputs ----
            # DMA emission order = execution order on the shared DMA engines:
            # interleave weight chunks with the first x segment so the first
            # matmul unblocks after ~1 MB, then stream remaining segments.
            mts = [cpool.tile([128, 3, D], bf16, name=f"mts{o}") for o in range(NCC)]
            segcols = [min(STRIDE * (_SEGENDS[s] - SEGSTARTS[s]) + 129,
                           XCOLS - STRIDE * SEGSTARTS[s]) for s in range(NSEG)]
            xsegs = [cpool.tile([128, NCC, segcols[s]], bf16, name=f"xseg{s}")
                     for s in range(NSEG)]

            x_r = x_d.rearrange("(o p) l -> p o l", p=128)

            # first segment fine-grained (per ic-chunk) so the first matmul
            # unblocks early; later segments as one bundled DMA each
            for o in range(NCC):
                nc.sync.dma_start(mts[o][:], mt_d[128 * o:128 * (o + 1)])
                nc.sync.dma_start(xsegs[0][:, o], x_r[:, o, :segcols[0]])

            rts = cpool.tile([128, 2, RCOLS], bf16, name="rts")
            nc.sync.dma_start(rts[:], rt_d[:])
            biast = cpool.tile([128, NCC], f32, name="biast")
            nc.sync.dma_start(biast[:], bias_d.rearrange("(o p) -> p o", p=128))

            for s in range(1, NSEG):
                c0 = STRIDE * SEGSTARTS[s]
                nc.sync.dma_start(xsegs[s][:], x_r[:, :, c0:c0 + segcols[s]])

            zs = [None] * NJ
            pair_psum = {}
            out_r = out_d.rearrange("(cc p) l -> p cc l", p=128)

            # warm-up matmuls on a zeroed scratch tile: ramps the PE clock
            # (HAM / p-state) out of its cold state while the first DMAs land
            # sized to cover plausible real-HW first-data latency (~2us with
            # parallel DGE queues) without delaying the first real matmul;
            # sim total is invariant for NWARM in 0..160
            NWARM = 40
            wsb = cpool.tile([128, 64], bf16, name="wsb")
            nc.vector.memset(wsb[:], 0.0)
            wps = zmm.tile([128, D], f32, tag="zmm", name="wps")
            for _ in range(NWARM):
                nc.tensor.matmul(wps[:64, :64], lhsT=wsb[:], rhs=wsb[:],
                                 start=True, stop=True)

            seg_of = {J: s for s in range(NSEG)
                      for J in range(SEGSTARTS[s], _SEGENDS[s])}

            def emit_main(J):
                # z tile J covers u in [120J-4, 120J+124); SBUF col = u + XPAD_L
                s = seg_of[J]
                xt = xsegs[s]
                c0 = STRIDE * (J - SEGSTARTS[s])
                ze_ps = zmm.tile([128, D], f32, tag="zmm", name="ze_ps")
                for o in range(NCC):
                    nc.tensor.matmul(
                        ze_ps, lhsT=xt[:, o, c0:c0 + 128], rhs=mts[o][:, 1],
                        start=(o == 0), stop=(o == NCC - 1),
                    )
                zo_ps = zmm.tile([128, D], f32, tag="zmm", name="zo_ps")
                for o in range(NCC):
                    nc.tensor.matmul(
                        zo_ps, lhsT=xt[:, o, c0:c0 + 128], rhs=mts[o][:, 0],
                        start=(o == 0), stop=False,
                    )
                for o in range(NCC):
                    nc.tensor.matmul(
                        zo_ps, lhsT=xt[:, o, c0 + 1:c0 + 129], rhs=mts[o][:, 2],
                        start=False, stop=(o == NCC - 1),
                    )
                ze_sb = zpool.tile([128, D], bf16, tag="ze", name="ze_sb")
                zo_sb = zpool.tile([128, D], bf16, tag="zo", name="zo_sb")
                if J == NJ - 1:
                    # last tile: run the two copies on different engines in
                    # parallel -- they gate the kernel's final AA matmuls
                    nc.vector.tensor_copy(out=ze_sb[:], in_=ze_ps[:])
                    nc.scalar.copy(out=zo_sb[:], in_=zo_ps[:])
                elif J % 2 == 0:
                    nc.vector.tensor_copy(out=ze_sb[:], in_=ze_ps[:])
                    nc.vector.tensor_copy(out=zo_sb[:], in_=zo_ps[:])
                else:
                    nc.scalar.copy(out=ze_sb[:], in_=ze_ps[:])
                    nc.scalar.copy(out=zo_sb[:], in_=zo_ps[:])
                zs[J] = (ze_sb, zo_sb)

            # out-tile grouping: pairs of J share one PSUM bank row; the last
            # three go solo so each one's copies+DMA overlap the remaining
            # PE work instead of trailing the kernel
            PAIRS = [(a, a + 1) for a in range(0, NJ - 1, 2)] + [(NJ - 1,)]
            pair_of = {J: (p, grp.index(J), grp) for p, grp in enumerate(PAIRS)
                       for J in grp}

            def emit_aa(J):
                p, half, grp = pair_of[J]
                if half == 0:
                    pair_psum[p] = [
                        aamm.tile([128, 512], f32, tag="aa", name=f"aa_ps{cc}")
                        for cc in range(NCC)
                    ]
                # last tile: only 32 of the 240 interleaved out cols are real
                mmcols = min(RCOLS, LOUT - RCOLS * J)
                ze_sb, zo_sb = zs[J]
                for cc in range(NCC):
                    dst = pair_psum[p][cc][:, RCOLS * half:RCOLS * half + mmcols]
                    nc.tensor.matmul(
                        dst, lhsT=ze_sb[:, 128 * cc:128 * (cc + 1)],
                        rhs=rts[:, 0, :mmcols], start=True, stop=False,
                    )
                    nc.tensor.matmul(
                        dst, lhsT=zo_sb[:, 128 * cc:128 * (cc + 1)],
                        rhs=rts[:, 1, :mmcols], start=False, stop=True,
                    )
                zs[J] = None
                if half == len(grp) - 1:
                    lbase = RCOLS * grp[0]
                    ncols = min(RCOLS * len(grp), LOUT - lbase)
                    osb = opool.tile([128, NCC, 2 * RCOLS], f32,
                                     tag="osb", name="osb")
                    for cc in range(NCC):
                        if (p * NCC + cc) % 2 == 0:
                            nc.scalar.activation(
                                osb[:, cc, :ncols], pair_psum[p][cc][:, :ncols],
                                mybir.ActivationFunctionType.Identity,
                                bias=biast[:, cc:cc + 1], scale=1.0,
                            )
                        else:
                            nc.vector.tensor_tensor(
                                osb[:, cc, :ncols], pair_psum[p][cc][:, :ncols],
                                biast[:, cc:cc + 1].to_broadcast((128, ncols)),
                                mybir.AluOpType.add,
                            )
                    nc.sync.dma_start(
                        out_r[:, :, lbase:lbase + ncols], osb[:, :, :ncols]
                    )
                    del pair_psum[p]

            # software-pipelined emission: AA(J-1) after main(J) so the PE
            # never waits on the DVE z copies.  The skew collapses at the
            # last J so the second-to-last pair's copies + out DMAs overlap
            # main(NJ-1) instead of trailing the whole kernel.
            for J in range(NJ - 1):
                emit_main(J)
                if J >= 1:
                    emit_aa(J - 1)
            emit_aa(NJ - 2)
            emit_main(NJ - 1)
            emit_aa(NJ - 1)

    nc.compile()
    return nc


def _host_weights(conv_w, conv_b, aa_kernel, proj_w, proj_b):
    aa = np.asarray(aa_kernel, np.float32)
    proj_w = np.asarray(proj_w, np.float32)
    # fold the projection into the three polyphase matrices
    m = [proj_w @ np.asarray(conv_w, np.float32)[:, :, k] for k in range(3)]
    mt_np = np.stack([mk.T for mk in m], axis=1).astype(BF16)  # [ic, 3, oc]

    u = np.arange(128)[:, None]
    l = np.arange(RCOLS)[None, :]
    te = 2 * u - l
    to = 2 * u - l + 1
    r_e = np.where((te >= 0) & (te < KSIZE), aa[np.clip(te, 0, KSIZE - 1)], 0.0)
    r_o = np.where((to >= 0) & (to < KSIZE), aa[np.clip(to, 0, KSIZE - 1)], 0.0)
    rt_np = np.stack([r_e, r_o], axis=1).astype(BF16)  # [128, 2, 240]

    bias_np = (aa.sum() * (proj_w @ np.asarray(conv_b, np.float32))
               + np.asarray(proj_b, np.float32)).astype(np.float32)
    return mt_np, rt_np, bias_np


def kernel(x, conv_w, conv_b, aa_kernel, proj_w, proj_b):
    if "nc" not in _CACHE:
        _CACHE["nc"] = _build_bass()
    nc = _CACHE["nc"]

    mt_np, rt_np, bias_np = _host_weights(conv_w, conv_b, aa_kernel, proj_w, proj_b)
    x = np.asarray(x, np.float32)
    xpad = np.zeros((B, D, XCOLS), BF16)
    xpad[:, :, XPAD_L:XPAD_L + L] = x.astype(BF16)
    in_maps = [
        {"x": xpad[b], "mt": mt_np, "rt": rt_np, "bias": bias_np}
        for b in range(B)
    ]
    try:
        res = run_bass_kernel_spmd(nc, in_maps, core_ids=list(range(B)))
    except ModuleNotFoundError:
        # axon tunnel without NTFF profiling hooks + BASS_TRACE set in the
        # environment: retry untraced
        import os
        os.environ["BASS_NEVER_TRACE"] = "1"
        res = run_bass_kernel_spmd(nc, in_maps, core_ids=list(range(B)))
    _CACHE["last_results"] = res
    return np.stack([r["out"] for r in res.results], axis=0)
